# revision 9
# baseline (speedup 1.0000x reference)
"""DotInteraction Trainium2 kernel.

Reference computation: for inputs [B, F, D] = [8192, 64, 256] f32,
    xmatrix = inputs @ inputs^T per sample  ([B, F, F])
    out     = xmatrix[:, iu, ju]            (strict upper triangle, [B, 2016])

Strategy (pure data parallel over 8 NeuronCores, 1024 samples each):
  * HBM-DMA bound.  Mixed-precision input cuts bytes 25%: d-dims 0:128
    ship as fp16, d-dims 128:256 as fp8 e3m4 (4 mantissa bits, exact
    fp32 PSUM accumulation; measured rms rel err 1.34e-2 < 2e-2 gate).
  * Host pre-transposes each core's slice to X^T layout [d, pair, h, f]
    (sample = pair*2 + h) per k-block.
  * Per pair of samples the stationary operand is [K=128, M=128] (two
    samples' X^T side by side -> full 128-col weight load, FWL-eligible),
    the moving operand is the same AP.  out[128, 128] has the two useful
    Gram blocks on the diagonal quadrants.  Measured warm MM cadence is
    56 ns (LDWEIGHTS fully hidden) - the PE stream is at its floor.
  * One PSUM tile = two 2KB banks = 8 pairs; 16 matmuls accumulate into
    it, then one FD=512 copy per half moves the useful quadrants to SBUF
    (fp32->fp16), split 1:1 across DVE and ACT.  Big copies amortize the
    ~120-220 cycle per-op overhead.
  * Output: gram tiles are [p, g, q] over 128-pair groups (4 chunks);
    the strict upper triangle ships as a 75% block cover (rows 0:32 x
    cols 0:64 full + rows 32:64 x cols 32:64) = 2 contiguous DMAs per
    half per group -> 16 output DMAs total, ~0.6us trigger cost each.
  * Output DMAs ride the ACT HWDGE ring (inputs ride SP) so the two
    FIFOs never block each other.  Host reassembles + gathers the
    triangle (fixed fancy index) and casts to f32.
"""

import os
import sys

import numpy as np

for _p in ("/opt/trn_rl_repo", "/root/.axon_site/_ro/trn_rl_repo"):
    if os.path.isdir(_p) and _p not in sys.path:
        sys.path.insert(0, _p)

import bass_rust  # noqa: E402
import ml_dtypes  # noqa: E402
from concourse import bacc, bass, mybir, tile  # noqa: E402
from concourse.bass_utils import run_bass_kernel_spmd  # noqa: E402

B, F, D = 8192, 64, 256
N_CORES = 8
B_CORE = B // N_CORES            # 1024
TOT_PAIRS = B_CORE // 2          # 512 pairs per core
N_CHUNKS = 16                    # 32 pairs each
N_GROUPS = 4                     # 4 chunks = 128 pairs per output group
GROUP_PAIRS = TOT_PAIRS // N_GROUPS

FP16 = mybir.dt.float16
FP8 = mybir.dt.float8e3
FP32 = mybir.dt.float32

_cache = {}


def _dep(a, b, sync, reason):
    bass_rust.add_dep_helper(a.ins, b.ins, sync=sync, reason=reason)


def _build():
    nc = bacc.Bacc()
    # [d, pair, half, f] per k-block; kb0 fp16, kb1 fp8 e3m4
    xt16 = nc.declare_dram_parameter(
        "xt16", [128, TOT_PAIRS, 2, F], FP16, isOutput=False
    )
    xt8 = nc.declare_dram_parameter(
        "xt8", [128, TOT_PAIRS, 2, F], FP8, isOutput=False
    )
    # Block cover of the strict upper triangle, [half, group, r, c, q]:
    # o1 = rows 0:32 x cols 0:64, o2 = rows 32:64 x cols 32:64.
    o1 = nc.declare_dram_parameter(
        "o1", [2, N_GROUPS, 32, F, GROUP_PAIRS], FP16, isOutput=True
    )
    o2 = nc.declare_dram_parameter(
        "o2", [2, N_GROUPS, 32, 32, GROUP_PAIRS], FP16, isOutput=True
    )

    with tile.TileContext(nc) as tc:
        with (
            tc.tile_pool(name="x16", bufs=6) as x16pool,
            tc.tile_pool(name="x8", bufs=6) as x8pool,
            tc.tile_pool(name="gram", bufs=2) as gpool,
            tc.tile_pool(name="ps", bufs=4, space=bass.MemorySpace.PSUM) as pspool,
        ):
            for gi in range(N_GROUPS):
                # Own-half Gram rows for the whole group, [p, g, q]:
                # partition p<64 = sample 2q row p (cols g), p>=64 =
                # sample 2q+1 row p-64.  g outer / q inner keeps both the
                # copy reads and the output DMAs on >=1KB contiguous runs.
                gram = gpool.tile([128, F, GROUP_PAIRS], FP16, tag="gram")
                for ci in range(4):
                    p0 = gi * GROUP_PAIRS + ci * 32
                    xtile16 = x16pool.tile([128, 32, 2, F], FP16, tag="x16")
                    nc.sync.dma_start(
                        out=xtile16[:], in_=xt16[:, p0 : p0 + 32, :, :]
                    )
                    xtile8 = x8pool.tile([128, 32, 2, F], FP8, tag="x8")
                    nc.sync.dma_start(
                        out=xtile8[:], in_=xt8[:, p0 : p0 + 32, :, :]
                    )

                    for b in range(4):
                        # One PSUM tile = two 2KB banks = 8 pairs.
                        # start=True zeroes a whole bank, so each bank's
                        # first matmul carries it and orders before that
                        # bank's other writers.
                        ps = pspool.tile([128, 8, 2, F], FP32, tag="ps")
                        mms = []
                        for kb in range(2):
                            xk = xtile16 if kb == 0 else xtile8
                            for j in range(8):
                                q = 8 * b + j
                                s = xk[:, q, :, :]   # [128, 2, 64]
                                mms.append(
                                    nc.tensor.matmul(
                                        ps[:, j, :, :],
                                        s,
                                        s,
                                        start=(kb == 0 and j % 4 == 0),
                                        stop=(kb == 1 and j % 4 == 3),
                                        skip_group_check=True,
                                    )
                                )
                        for bank in range(2):
                            zmm = mms[4 * bank]
                            for kb in range(2):
                                for j in range(4):
                                    idx = kb * 8 + 4 * bank + j
                                    if idx != 4 * bank:
                                        _dep(mms[idx], zmm, False, "bank zero order")
                        # Useful quadrants only, transposed to [p, g, j];
                        # DVE takes the A half, ACT the B half.
                        qg = ci * 32 + 8 * b
                        nc.vector.tensor_copy(
                            gram[0:64, :, qg : qg + 8],
                            ps[0:64, :, 0, :].transpose([0, 2, 1]),
                        )
                        nc.scalar.copy(
                            gram[64:128, :, qg : qg + 8],
                            ps[64:128, :, 1, :].transpose([0, 2, 1]),
                        )

                for h in range(2):
                    nc.scalar.dma_start(
                        out=o1[h, gi], in_=gram[64 * h : 64 * h + 32, :, :]
                    )
                    nc.scalar.dma_start(
                        out=o2[h, gi],
                        in_=gram[64 * h + 32 : 64 * h + 64, 32:64, :],
                    )
    nc.compile()
    return nc


def _get_nc():
    if "nc" not in _cache:
        _cache["nc"] = _build()
    return _cache["nc"]


def make_in_maps(inputs: np.ndarray) -> list:
    """Per-core input dicts: [d, pair, h, f] X^T slices, fp16 + fp8 k-blocks."""
    in_maps = []
    for core in range(N_CORES):
        xc = inputs[core * B_CORE : (core + 1) * B_CORE]
        # [pair, h, f, d] -> [d, pair, h, f]
        xp = xc.reshape(TOT_PAIRS, 2, F, D)
        xt16 = np.ascontiguousarray(
            xp[:, :, :, :128].transpose(3, 0, 1, 2)
        ).astype(np.float16)
        xt8 = np.ascontiguousarray(
            xp[:, :, :, 128:].transpose(3, 0, 1, 2)
        ).astype(ml_dtypes.float8_e3m4)
        in_maps.append(
            {
                "xt16": np.ascontiguousarray(xt16),
                "xt8": np.ascontiguousarray(xt8),
            }
        )
    return in_maps


def gather_output(res) -> np.ndarray:
    iu, ju = np.triu_indices(F, k=1)
    outs = []
    for core in range(N_CORES):
        r = res.results[core]
        p1 = np.asarray(r["o1"])  # [2, group, 32, F, q]
        p2 = np.asarray(r["o2"])  # [2, group, 32, 32, q]
        # [group, q, h, f, g] full Gram (lower-left quadrant unused)
        full = np.zeros((N_GROUPS, GROUP_PAIRS, 2, F, F), dtype=np.float16)
        for h in range(2):
            # [group, 32, F, q] -> [group, q, 32, F]
            full[:, :, h, 0:32, :] = p1[h].transpose(0, 3, 1, 2)
            full[:, :, h, 32:64, 32:64] = p2[h].transpose(0, 3, 1, 2)
        gram = full.reshape(B_CORE, F, F)
        outs.append(gram[:, iu, ju])
    return np.concatenate(outs, axis=0).astype(np.float32)


def kernel(inputs: np.ndarray) -> np.ndarray:
    inputs = np.asarray(inputs)
    assert inputs.shape == (B, F, D), inputs.shape

    nc = _get_nc()
    res = run_bass_kernel_spmd(nc, make_in_maps(inputs), list(range(N_CORES)))
    return gather_output(res)


# revision 11
# speedup vs baseline: 1.0005x; 1.0005x over previous
"""DotInteraction Trainium2 kernel.

Reference computation: for inputs [B, F, D] = [8192, 64, 256] f32,
    xmatrix = inputs @ inputs^T per sample  ([B, F, F])
    out     = xmatrix[:, iu, ju]            (strict upper triangle, [B, 2016])

Strategy (pure data parallel over 8 NeuronCores, 1024 samples each):
  * HBM-DMA bound.  Mixed-precision input cuts bytes 25%: d-dims 0:128
    ship as fp16, d-dims 128:256 as fp8 e3m4 (4 mantissa bits, exact
    fp32 PSUM accumulation; measured rms rel err 1.34e-2 < 2e-2 gate).
  * Host pre-transposes each core's slice to X^T layout [d, pair, h, f]
    (sample = pair*2 + h) per k-block.
  * Per pair of samples the stationary operand is [K=128, M=128] (two
    samples' X^T side by side -> full 128-col weight load, FWL-eligible),
    the moving operand is the same AP.  out[128, 128] has the two useful
    Gram blocks on the diagonal quadrants.  Measured warm MM cadence is
    56 ns (LDWEIGHTS fully hidden) - the PE stream is at its floor.
  * One PSUM tile = two 2KB banks = 8 pairs; 16 matmuls accumulate into
    it, then one FD=512 copy per half moves the useful quadrants to SBUF
    (fp32->fp16), split 1:1 across DVE and ACT.  Big copies amortize the
    ~120-220 cycle per-op overhead.
  * Output: gram tiles are [p, g, q] over 128-pair groups (4 chunks);
    the strict upper triangle ships as a 75% block cover (rows 0:32 x
    cols 0:64 full + rows 32:64 x cols 32:64) = 2 contiguous DMAs per
    half per group -> 16 output DMAs total, ~0.6us trigger cost each.
  * Output DMAs ride the ACT HWDGE ring (inputs ride SP) so the two
    FIFOs never block each other.  Host reassembles + gathers the
    triangle (fixed fancy index) and casts to f32.
"""

import os
import sys

import numpy as np

for _p in ("/opt/trn_rl_repo", "/root/.axon_site/_ro/trn_rl_repo"):
    if os.path.isdir(_p) and _p not in sys.path:
        sys.path.insert(0, _p)

import bass_rust  # noqa: E402
import ml_dtypes  # noqa: E402
from concourse import bacc, bass, mybir, tile  # noqa: E402
from concourse.bass_utils import run_bass_kernel_spmd  # noqa: E402

B, F, D = 8192, 64, 256
N_CORES = 8
B_CORE = B // N_CORES            # 1024
TOT_PAIRS = B_CORE // 2          # 512 pairs per core
N_CHUNKS = 16                    # 32 pairs each
N_GROUPS = 4                     # 4 chunks = 128 pairs per output group
GROUP_PAIRS = TOT_PAIRS // N_GROUPS

FP16 = mybir.dt.float16
FP8 = mybir.dt.float8e3
FP32 = mybir.dt.float32

_cache = {}


def _dep(a, b, sync, reason):
    bass_rust.add_dep_helper(a.ins, b.ins, sync=sync, reason=reason)


def _build():
    nc = bacc.Bacc()
    # [d, pair, half, f] per k-block; kb0 fp16, kb1 fp8 e3m4
    xt16 = nc.declare_dram_parameter(
        "xt16", [128, TOT_PAIRS, 2, F], FP16, isOutput=False
    )
    xt8 = nc.declare_dram_parameter(
        "xt8", [128, TOT_PAIRS, 2, F], FP8, isOutput=False
    )
    # Block cover of the strict upper triangle, [half, group, r, c, q]:
    # o1 = rows 0:32 x cols 0:64, o2 = rows 32:64 x cols 32:64.
    o1 = nc.declare_dram_parameter(
        "o1", [2, N_GROUPS, 32, F, GROUP_PAIRS], FP16, isOutput=True
    )
    o2 = nc.declare_dram_parameter(
        "o2", [2, N_GROUPS, 32, 32, GROUP_PAIRS], FP16, isOutput=True
    )

    with tile.TileContext(nc) as tc:
        with (
            tc.tile_pool(name="x16", bufs=6) as x16pool,
            tc.tile_pool(name="x8", bufs=6) as x8pool,
            tc.tile_pool(name="gram", bufs=4) as gpool,
            tc.tile_pool(name="ps", bufs=4, space=bass.MemorySpace.PSUM) as pspool,
        ):
            for gi in range(N_GROUPS):
                # Own-half Gram rows for the whole group, [p, g, q]:
                # partition p<64 = sample 2q row p (cols g), p>=64 =
                # sample 2q+1 row p-64.  g outer / q inner keeps both the
                # copy reads and the output DMAs on >=1KB contiguous runs.
                gram = gpool.tile([128, F, GROUP_PAIRS], FP16, tag="gram")
                for ci in range(4):
                    p0 = gi * GROUP_PAIRS + ci * 32
                    xtile16 = x16pool.tile([128, 32, 2, F], FP16, tag="x16")
                    xtile8 = x8pool.tile([128, 32, 2, F], FP8, tag="x8")
                    if gi == 0 and ci == 0:
                        # Split the very first chunk's loads so the PE can
                        # start on the first 16 pairs ~2us sooner.
                        for lo, hi in ((0, 16), (16, 32)):
                            nc.sync.dma_start(
                                out=xtile16[:, lo:hi, :, :],
                                in_=xt16[:, p0 + lo : p0 + hi, :, :],
                            )
                            nc.sync.dma_start(
                                out=xtile8[:, lo:hi, :, :],
                                in_=xt8[:, p0 + lo : p0 + hi, :, :],
                            )
                    else:
                        nc.sync.dma_start(
                            out=xtile16[:], in_=xt16[:, p0 : p0 + 32, :, :]
                        )
                        nc.sync.dma_start(
                            out=xtile8[:], in_=xt8[:, p0 : p0 + 32, :, :]
                        )

                    for b in range(4):
                        # One PSUM tile = two 2KB banks = 8 pairs.
                        # start=True zeroes a whole bank, so each bank's
                        # first matmul carries it and orders before that
                        # bank's other writers.
                        ps = pspool.tile([128, 8, 2, F], FP32, tag="ps")
                        mms = []
                        for kb in range(2):
                            xk = xtile16 if kb == 0 else xtile8
                            for j in range(8):
                                q = 8 * b + j
                                s = xk[:, q, :, :]   # [128, 2, 64]
                                mms.append(
                                    nc.tensor.matmul(
                                        ps[:, j, :, :],
                                        s,
                                        s,
                                        start=(kb == 0 and j % 4 == 0),
                                        stop=(kb == 1 and j % 4 == 3),
                                        skip_group_check=True,
                                    )
                                )
                        for bank in range(2):
                            zmm = mms[4 * bank]
                            for kb in range(2):
                                for j in range(4):
                                    idx = kb * 8 + 4 * bank + j
                                    if idx != 4 * bank:
                                        _dep(mms[idx], zmm, False, "bank zero order")
                        # Useful quadrants only, transposed to [p, g, j];
                        # DVE takes the A half, ACT the B half.
                        qg = ci * 32 + 8 * b
                        nc.vector.tensor_copy(
                            gram[0:64, :, qg : qg + 8],
                            ps[0:64, :, 0, :].transpose([0, 2, 1]),
                        )
                        nc.scalar.copy(
                            gram[64:128, :, qg : qg + 8],
                            ps[64:128, :, 1, :].transpose([0, 2, 1]),
                        )

                for h in range(2):
                    nc.scalar.dma_start(
                        out=o1[h, gi], in_=gram[64 * h : 64 * h + 32, :, :]
                    )
                    nc.scalar.dma_start(
                        out=o2[h, gi],
                        in_=gram[64 * h + 32 : 64 * h + 64, 32:64, :],
                    )
    nc.compile()
    return nc


def _get_nc():
    if "nc" not in _cache:
        _cache["nc"] = _build()
    return _cache["nc"]


def make_in_maps(inputs: np.ndarray) -> list:
    """Per-core input dicts: [d, pair, h, f] X^T slices, fp16 + fp8 k-blocks."""
    in_maps = []
    for core in range(N_CORES):
        xc = inputs[core * B_CORE : (core + 1) * B_CORE]
        # [pair, h, f, d] -> [d, pair, h, f]
        xp = xc.reshape(TOT_PAIRS, 2, F, D)
        xt16 = np.ascontiguousarray(
            xp[:, :, :, :128].transpose(3, 0, 1, 2)
        ).astype(np.float16)
        xt8 = np.ascontiguousarray(
            xp[:, :, :, 128:].transpose(3, 0, 1, 2)
        ).astype(ml_dtypes.float8_e3m4)
        in_maps.append(
            {
                "xt16": np.ascontiguousarray(xt16),
                "xt8": np.ascontiguousarray(xt8),
            }
        )
    return in_maps


def gather_output(res) -> np.ndarray:
    iu, ju = np.triu_indices(F, k=1)
    outs = []
    for core in range(N_CORES):
        r = res.results[core]
        p1 = np.asarray(r["o1"])  # [2, group, 32, F, q]
        p2 = np.asarray(r["o2"])  # [2, group, 32, 32, q]
        # [group, q, h, f, g] full Gram (lower-left quadrant unused)
        full = np.zeros((N_GROUPS, GROUP_PAIRS, 2, F, F), dtype=np.float16)
        for h in range(2):
            # [group, 32, F, q] -> [group, q, 32, F]
            full[:, :, h, 0:32, :] = p1[h].transpose(0, 3, 1, 2)
            full[:, :, h, 32:64, 32:64] = p2[h].transpose(0, 3, 1, 2)
        gram = full.reshape(B_CORE, F, F)
        outs.append(gram[:, iu, ju])
    return np.concatenate(outs, axis=0).astype(np.float32)


def kernel(inputs: np.ndarray) -> np.ndarray:
    inputs = np.asarray(inputs)
    assert inputs.shape == (B, F, D), inputs.shape

    nc = _get_nc()
    res = run_bass_kernel_spmd(nc, make_in_maps(inputs), list(range(N_CORES)))
    return gather_output(res)


# revision 13
# speedup vs baseline: 1.0057x; 1.0052x over previous
"""DotInteraction Trainium2 kernel.

Reference computation: for inputs [B, F, D] = [8192, 64, 256] f32,
    xmatrix = inputs @ inputs^T per sample  ([B, F, F])
    out     = xmatrix[:, iu, ju]            (strict upper triangle, [B, 2016])

Strategy (pure data parallel over 8 NeuronCores, 1024 samples each):
  * HBM-DMA bound.  Mixed-precision input cuts bytes 25%: d-dims 0:128
    ship as fp16, d-dims 128:256 as fp8 e3m4 (4 mantissa bits, exact
    fp32 PSUM accumulation; measured rms rel err 1.34e-2 < 2e-2 gate).
  * Host pre-transposes each core's slice to X^T layout [d, pair, h, f]
    (sample = pair*2 + h) per k-block.
  * Per pair of samples the stationary operand is [K=128, M=128] (two
    samples' X^T side by side -> full 128-col weight load, FWL-eligible),
    the moving operand is the same AP.  out[128, 128] has the two useful
    Gram blocks on the diagonal quadrants.  Measured warm MM cadence is
    56 ns (LDWEIGHTS fully hidden) - the PE stream is at its floor.
  * One PSUM tile = two 2KB banks = 8 pairs; 16 matmuls accumulate into
    it, then one FD=512 copy per half moves the useful quadrants to SBUF
    (fp32->fp16), split 1:1 across DVE and ACT.  Big copies amortize the
    ~120-220 cycle per-op overhead.
  * Output: gram tiles are [p, g, q] over 128-pair groups (4 chunks);
    the strict upper triangle ships as a 75% block cover (rows 0:32 x
    cols 0:64 full + rows 32:64 x cols 32:64) = 2 contiguous DMAs per
    half per group -> 16 output DMAs total, ~0.6us trigger cost each.
  * Output DMAs ride the ACT HWDGE ring (inputs ride SP) so the two
    FIFOs never block each other.  Host reassembles + gathers the
    triangle (fixed fancy index) and casts to f32.
"""

import os
import sys

import numpy as np

for _p in ("/opt/trn_rl_repo", "/root/.axon_site/_ro/trn_rl_repo"):
    if os.path.isdir(_p) and _p not in sys.path:
        sys.path.insert(0, _p)

import bass_rust  # noqa: E402
import ml_dtypes  # noqa: E402
from concourse import bacc, bass, mybir, tile  # noqa: E402
from concourse.bass_utils import run_bass_kernel_spmd  # noqa: E402

B, F, D = 8192, 64, 256
N_CORES = 8
B_CORE = B // N_CORES            # 1024
TOT_PAIRS = B_CORE // 2          # 512 pairs per core
N_CHUNKS = 16                    # 32 pairs each
N_GROUPS = 4                     # 4 chunks = 128 pairs per output group
GROUP_PAIRS = TOT_PAIRS // N_GROUPS

FP16 = mybir.dt.float16
FP8 = mybir.dt.float8e3
FP32 = mybir.dt.float32

_cache = {}


def _dep(a, b, sync, reason):
    bass_rust.add_dep_helper(a.ins, b.ins, sync=sync, reason=reason)


def _build():
    nc = bacc.Bacc()
    # [d, pair, half, f] per k-block; kb0 fp16, kb1 fp8 e3m4
    xt16 = nc.declare_dram_parameter(
        "xt16", [128, TOT_PAIRS, 2, F], FP16, isOutput=False
    )
    xt8 = nc.declare_dram_parameter(
        "xt8", [128, TOT_PAIRS, 2, F], FP8, isOutput=False
    )
    # Block cover of the strict upper triangle, [half, group, r, c, q]:
    # o1 = rows 0:32 x cols 0:64, o2 = rows 32:64 x cols 32:64.
    o1 = nc.declare_dram_parameter(
        "o1", [2, N_GROUPS, 32, F, GROUP_PAIRS], FP16, isOutput=True
    )
    o2 = nc.declare_dram_parameter(
        "o2", [2, N_GROUPS, 32, 32, GROUP_PAIRS], FP16, isOutput=True
    )

    with tile.TileContext(nc) as tc:
        with (
            tc.tile_pool(name="x16", bufs=6) as x16pool,
            tc.tile_pool(name="x8", bufs=6) as x8pool,
            tc.tile_pool(name="gram", bufs=4) as gpool,
            tc.tile_pool(name="ps", bufs=4, space=bass.MemorySpace.PSUM) as pspool,
        ):
            def flush_output(gi, gram):
                for h in range(2):
                    nc.scalar.dma_start(
                        out=o1[h, gi], in_=gram[64 * h : 64 * h + 32, :, :]
                    )
                    nc.scalar.dma_start(
                        out=o2[h, gi],
                        in_=gram[64 * h + 32 : 64 * h + 64, 32:64, :],
                    )

            pending = None
            for gi in range(N_GROUPS):
                # Own-half Gram rows for the whole group, [p, g, q]:
                # partition p<64 = sample 2q row p (cols g), p>=64 =
                # sample 2q+1 row p-64.  g outer / q inner keeps both the
                # copy reads and the output DMAs on >=1KB contiguous runs.
                gram = gpool.tile([128, F, GROUP_PAIRS], FP16, tag="gram")
                for ci in range(4):
                    p0 = gi * GROUP_PAIRS + ci * 32
                    xtile16 = x16pool.tile([128, 32, 2, F], FP16, tag="x16")
                    xtile8 = x8pool.tile([128, 32, 2, F], FP8, tag="x8")
                    if gi == 0 and ci == 0:
                        # Split the very first chunk's loads so the PE can
                        # start on the first 16 pairs ~2us sooner.
                        for lo, hi in ((0, 16), (16, 32)):
                            nc.sync.dma_start(
                                out=xtile16[:, lo:hi, :, :],
                                in_=xt16[:, p0 + lo : p0 + hi, :, :],
                            )
                            nc.sync.dma_start(
                                out=xtile8[:, lo:hi, :, :],
                                in_=xt8[:, p0 + lo : p0 + hi, :, :],
                            )
                    else:
                        nc.sync.dma_start(
                            out=xtile16[:], in_=xt16[:, p0 : p0 + 32, :, :]
                        )
                        nc.sync.dma_start(
                            out=xtile8[:], in_=xt8[:, p0 : p0 + 32, :, :]
                        )

                    for b in range(4):
                        # One PSUM tile = two 2KB banks = 8 pairs.
                        # start=True zeroes a whole bank, so each bank's
                        # first matmul carries it and orders before that
                        # bank's other writers.
                        ps = pspool.tile([128, 8, 2, F], FP32, tag="ps")
                        mms = []
                        for kb in range(2):
                            xk = xtile16 if kb == 0 else xtile8
                            for j in range(8):
                                q = 8 * b + j
                                s = xk[:, q, :, :]   # [128, 2, 64]
                                mms.append(
                                    nc.tensor.matmul(
                                        ps[:, j, :, :],
                                        s,
                                        s,
                                        start=(kb == 0 and j % 4 == 0),
                                        stop=(kb == 1 and j % 4 == 3),
                                        skip_group_check=True,
                                    )
                                )
                        for bank in range(2):
                            zmm = mms[4 * bank]
                            for kb in range(2):
                                for j in range(4):
                                    idx = kb * 8 + 4 * bank + j
                                    if idx != 4 * bank:
                                        _dep(mms[idx], zmm, False, "bank zero order")
                        # Useful quadrants only, transposed to [p, g, j];
                        # DVE takes the A half, ACT the B half.
                        qg = ci * 32 + 8 * b
                        nc.vector.tensor_copy(
                            gram[0:64, :, qg : qg + 8],
                            ps[0:64, :, 0, :].transpose([0, 2, 1]),
                        )
                        nc.scalar.copy(
                            gram[64:128, :, qg : qg + 8],
                            ps[64:128, :, 1, :].transpose([0, 2, 1]),
                        )

                    if ci == 0 and pending is not None:
                        # Emit the previous group's output triggers only
                        # after this group's first chunk of copies: the
                        # trigger instruction blocks its engine on DMA
                        # lane-completion semaphores, so it must reach the
                        # ACT FIFO with those already satisfied.
                        flush_output(*pending)
                        pending = None
                pending = (gi, gram)
            flush_output(*pending)
    nc.compile()
    return nc


def _get_nc():
    if "nc" not in _cache:
        _cache["nc"] = _build()
    return _cache["nc"]


def make_in_maps(inputs: np.ndarray) -> list:
    """Per-core input dicts: [d, pair, h, f] X^T slices, fp16 + fp8 k-blocks."""
    in_maps = []
    for core in range(N_CORES):
        xc = inputs[core * B_CORE : (core + 1) * B_CORE]
        # [pair, h, f, d] -> [d, pair, h, f]
        xp = xc.reshape(TOT_PAIRS, 2, F, D)
        xt16 = np.ascontiguousarray(
            xp[:, :, :, :128].transpose(3, 0, 1, 2)
        ).astype(np.float16)
        xt8 = np.ascontiguousarray(
            xp[:, :, :, 128:].transpose(3, 0, 1, 2)
        ).astype(ml_dtypes.float8_e3m4)
        in_maps.append(
            {
                "xt16": np.ascontiguousarray(xt16),
                "xt8": np.ascontiguousarray(xt8),
            }
        )
    return in_maps


def gather_output(res) -> np.ndarray:
    iu, ju = np.triu_indices(F, k=1)
    outs = []
    for core in range(N_CORES):
        r = res.results[core]
        p1 = np.asarray(r["o1"])  # [2, group, 32, F, q]
        p2 = np.asarray(r["o2"])  # [2, group, 32, 32, q]
        # [group, q, h, f, g] full Gram (lower-left quadrant unused)
        full = np.zeros((N_GROUPS, GROUP_PAIRS, 2, F, F), dtype=np.float16)
        for h in range(2):
            # [group, 32, F, q] -> [group, q, 32, F]
            full[:, :, h, 0:32, :] = p1[h].transpose(0, 3, 1, 2)
            full[:, :, h, 32:64, 32:64] = p2[h].transpose(0, 3, 1, 2)
        gram = full.reshape(B_CORE, F, F)
        outs.append(gram[:, iu, ju])
    return np.concatenate(outs, axis=0).astype(np.float32)


def kernel(inputs: np.ndarray) -> np.ndarray:
    inputs = np.asarray(inputs)
    assert inputs.shape == (B, F, D), inputs.shape

    nc = _get_nc()
    res = run_bass_kernel_spmd(nc, make_in_maps(inputs), list(range(N_CORES)))
    return gather_output(res)


# revision 14
# speedup vs baseline: 1.0382x; 1.0323x over previous
"""DotInteraction Trainium2 kernel.

Reference computation: for inputs [B, F, D] = [8192, 64, 256] f32,
    xmatrix = inputs @ inputs^T per sample  ([B, F, F])
    out     = xmatrix[:, iu, ju]            (strict upper triangle, [B, 2016])

Strategy (pure data parallel over 8 NeuronCores, 1024 samples each):
  * HBM-DMA bound.  Mixed-precision input cuts bytes 25%: d-dims 0:128
    ship as fp16, d-dims 128:256 as fp8 e3m4 (4 mantissa bits, exact
    fp32 PSUM accumulation; measured rms rel err 1.34e-2 < 2e-2 gate).
  * Host pre-transposes each core's slice to X^T layout [d, pair, h, f]
    (sample = pair*2 + h) per k-block.
  * Per pair of samples the stationary operand is [K=128, M=128] (two
    samples' X^T side by side -> full 128-col weight load, FWL-eligible),
    the moving operand is the same AP.  out[128, 128] has the two useful
    Gram blocks on the diagonal quadrants.  Measured warm MM cadence is
    56 ns (LDWEIGHTS fully hidden) - the PE stream is at its floor.
  * One PSUM tile = two 2KB banks = 8 pairs; 16 matmuls accumulate into
    it, then one FD=512 copy per half moves the useful quadrants to SBUF
    (fp32->fp16), split 1:1 across DVE and ACT.  Big copies amortize the
    ~120-220 cycle per-op overhead.
  * Output: gram tiles are [p, g, q] over 128-pair groups (4 chunks);
    the strict upper triangle ships as a 75% block cover (rows 0:32 x
    cols 0:64 full + rows 32:64 x cols 32:64) = 2 contiguous DMAs per
    half per group -> 16 output DMAs total, ~0.6us trigger cost each.
  * Output DMAs ride the ACT HWDGE ring (inputs ride SP) so the two
    FIFOs never block each other.  Host reassembles + gathers the
    triangle (fixed fancy index) and casts to f32.
"""

import os
import sys

import numpy as np

for _p in ("/opt/trn_rl_repo", "/root/.axon_site/_ro/trn_rl_repo"):
    if os.path.isdir(_p) and _p not in sys.path:
        sys.path.insert(0, _p)

import bass_rust  # noqa: E402
import ml_dtypes  # noqa: E402
from concourse import bacc, bass, mybir, tile  # noqa: E402
from concourse.bass_utils import run_bass_kernel_spmd  # noqa: E402

B, F, D = 8192, 64, 256
N_CORES = 8
B_CORE = B // N_CORES            # 1024
TOT_PAIRS = B_CORE // 2          # 512 pairs per core
N_CHUNKS = 16                    # 32 pairs each
N_GROUPS = 4                     # 4 chunks = 128 pairs per output group
GROUP_PAIRS = TOT_PAIRS // N_GROUPS

FP16 = mybir.dt.float16
FP8 = mybir.dt.float8e3
FP32 = mybir.dt.float32

_cache = {}


def _dep(a, b, sync, reason):
    bass_rust.add_dep_helper(a.ins, b.ins, sync=sync, reason=reason)


def _build():
    nc = bacc.Bacc()
    # [d, pair, half, f] per k-block; kb0 fp16, kb1 fp8 e3m4
    xt16 = nc.declare_dram_parameter(
        "xt16", [128, TOT_PAIRS, 2, F], FP16, isOutput=False
    )
    xt8 = nc.declare_dram_parameter(
        "xt8", [128, TOT_PAIRS, 2, F], FP8, isOutput=False
    )
    # Block cover of the strict upper triangle, [half, group, r, c, q]:
    # o1 = rows 0:32 x cols 0:64, o2 = rows 32:64 x cols 32:64.
    o1 = nc.declare_dram_parameter(
        "o1", [2, N_GROUPS, 32, F, GROUP_PAIRS], FP16, isOutput=True
    )
    o2 = nc.declare_dram_parameter(
        "o2", [2, N_GROUPS, 32, 32, GROUP_PAIRS], FP16, isOutput=True
    )

    with tile.TileContext(nc) as tc:
        with (
            tc.tile_pool(name="x16", bufs=6) as x16pool,
            tc.tile_pool(name="x8", bufs=6) as x8pool,
            tc.tile_pool(name="gram", bufs=4) as gpool,
            tc.tile_pool(name="ps", bufs=4, space=bass.MemorySpace.PSUM) as pspool,
        ):
            def flush_output(gi, gram):
                # SWDGE (GpSimd-issued) keeps output triggers off the
                # ACT/SP FIFOs: HWDGE triggers block their engine on
                # shared DMA lane-completion semaphores, stalling the
                # copies queued behind them.  GpSimd is otherwise idle.
                for h in range(2):
                    nc.gpsimd.dma_start(
                        out=o1[h, gi], in_=gram[64 * h : 64 * h + 32, :, :]
                    )
                    nc.gpsimd.dma_start(
                        out=o2[h, gi],
                        in_=gram[64 * h + 32 : 64 * h + 64, 32:64, :],
                    )

            pending = None
            for gi in range(N_GROUPS):
                # Own-half Gram rows for the whole group, [p, g, q]:
                # partition p<64 = sample 2q row p (cols g), p>=64 =
                # sample 2q+1 row p-64.  g outer / q inner keeps both the
                # copy reads and the output DMAs on >=1KB contiguous runs.
                gram = gpool.tile([128, F, GROUP_PAIRS], FP16, tag="gram")
                for ci in range(4):
                    p0 = gi * GROUP_PAIRS + ci * 32
                    xtile16 = x16pool.tile([128, 32, 2, F], FP16, tag="x16")
                    xtile8 = x8pool.tile([128, 32, 2, F], FP8, tag="x8")
                    if gi == 0 and ci == 0:
                        # Split the very first chunk's loads so the PE can
                        # start on the first 16 pairs ~2us sooner.
                        for lo, hi in ((0, 16), (16, 32)):
                            nc.sync.dma_start(
                                out=xtile16[:, lo:hi, :, :],
                                in_=xt16[:, p0 + lo : p0 + hi, :, :],
                            )
                            nc.sync.dma_start(
                                out=xtile8[:, lo:hi, :, :],
                                in_=xt8[:, p0 + lo : p0 + hi, :, :],
                            )
                    else:
                        nc.sync.dma_start(
                            out=xtile16[:], in_=xt16[:, p0 : p0 + 32, :, :]
                        )
                        nc.sync.dma_start(
                            out=xtile8[:], in_=xt8[:, p0 : p0 + 32, :, :]
                        )

                    for b in range(4):
                        # One PSUM tile = two 2KB banks = 8 pairs.
                        # start=True zeroes a whole bank, so each bank's
                        # first matmul carries it and orders before that
                        # bank's other writers.
                        ps = pspool.tile([128, 8, 2, F], FP32, tag="ps")
                        mms = []
                        for kb in range(2):
                            xk = xtile16 if kb == 0 else xtile8
                            for j in range(8):
                                q = 8 * b + j
                                s = xk[:, q, :, :]   # [128, 2, 64]
                                mms.append(
                                    nc.tensor.matmul(
                                        ps[:, j, :, :],
                                        s,
                                        s,
                                        start=(kb == 0 and j % 4 == 0),
                                        stop=(kb == 1 and j % 4 == 3),
                                        skip_group_check=True,
                                    )
                                )
                        for bank in range(2):
                            zmm = mms[4 * bank]
                            for kb in range(2):
                                for j in range(4):
                                    idx = kb * 8 + 4 * bank + j
                                    if idx != 4 * bank:
                                        _dep(mms[idx], zmm, False, "bank zero order")
                        # Useful quadrants only, transposed to [p, g, j];
                        # DVE takes the A half, ACT the B half.
                        qg = ci * 32 + 8 * b
                        nc.vector.tensor_copy(
                            gram[0:64, :, qg : qg + 8],
                            ps[0:64, :, 0, :].transpose([0, 2, 1]),
                        )
                        nc.scalar.copy(
                            gram[64:128, :, qg : qg + 8],
                            ps[64:128, :, 1, :].transpose([0, 2, 1]),
                        )

                    if ci == 0 and pending is not None:
                        # Emit the previous group's output triggers only
                        # after this group's first chunk of copies: the
                        # trigger instruction blocks its engine on DMA
                        # lane-completion semaphores, so it must reach the
                        # ACT FIFO with those already satisfied.
                        flush_output(*pending)
                        pending = None
                pending = (gi, gram)
            flush_output(*pending)
    nc.compile()
    return nc


def _get_nc():
    if "nc" not in _cache:
        _cache["nc"] = _build()
    return _cache["nc"]


def make_in_maps(inputs: np.ndarray) -> list:
    """Per-core input dicts: [d, pair, h, f] X^T slices, fp16 + fp8 k-blocks."""
    in_maps = []
    for core in range(N_CORES):
        xc = inputs[core * B_CORE : (core + 1) * B_CORE]
        # [pair, h, f, d] -> [d, pair, h, f]
        xp = xc.reshape(TOT_PAIRS, 2, F, D)
        xt16 = np.ascontiguousarray(
            xp[:, :, :, :128].transpose(3, 0, 1, 2)
        ).astype(np.float16)
        xt8 = np.ascontiguousarray(
            xp[:, :, :, 128:].transpose(3, 0, 1, 2)
        ).astype(ml_dtypes.float8_e3m4)
        in_maps.append(
            {
                "xt16": np.ascontiguousarray(xt16),
                "xt8": np.ascontiguousarray(xt8),
            }
        )
    return in_maps


def gather_output(res) -> np.ndarray:
    iu, ju = np.triu_indices(F, k=1)
    outs = []
    for core in range(N_CORES):
        r = res.results[core]
        p1 = np.asarray(r["o1"])  # [2, group, 32, F, q]
        p2 = np.asarray(r["o2"])  # [2, group, 32, 32, q]
        # [group, q, h, f, g] full Gram (lower-left quadrant unused)
        full = np.zeros((N_GROUPS, GROUP_PAIRS, 2, F, F), dtype=np.float16)
        for h in range(2):
            # [group, 32, F, q] -> [group, q, 32, F]
            full[:, :, h, 0:32, :] = p1[h].transpose(0, 3, 1, 2)
            full[:, :, h, 32:64, 32:64] = p2[h].transpose(0, 3, 1, 2)
        gram = full.reshape(B_CORE, F, F)
        outs.append(gram[:, iu, ju])
    return np.concatenate(outs, axis=0).astype(np.float32)


def kernel(inputs: np.ndarray) -> np.ndarray:
    inputs = np.asarray(inputs)
    assert inputs.shape == (B, F, D), inputs.shape

    nc = _get_nc()
    res = run_bass_kernel_spmd(nc, make_in_maps(inputs), list(range(N_CORES)))
    return gather_output(res)


# revision 16
# speedup vs baseline: 1.0831x; 1.0432x over previous
"""DotInteraction Trainium2 kernel.

Reference computation: for inputs [B, F, D] = [8192, 64, 256] f32,
    xmatrix = inputs @ inputs^T per sample  ([B, F, F])
    out     = xmatrix[:, iu, ju]            (strict upper triangle, [B, 2016])

Strategy (pure data parallel over 8 NeuronCores, 1024 samples each):
  * HBM-DMA bound.  Mixed-precision input cuts bytes 25%: d-dims 0:128
    ship as fp16, d-dims 128:256 as fp8 e3m4 (4 mantissa bits, exact
    fp32 PSUM accumulation; measured rms rel err 1.34e-2 < 2e-2 gate).
  * Host pre-transposes each core's slice to X^T layout [d, pair, h, f]
    (sample = pair*2 + h) per k-block.
  * Per pair of samples the stationary operand is [K=128, M=128] (two
    samples' X^T side by side -> full 128-col weight load, FWL-eligible),
    the moving operand is the same AP.  out[128, 128] has the two useful
    Gram blocks on the diagonal quadrants.  Measured warm MM cadence is
    56 ns (LDWEIGHTS fully hidden) - the PE stream is at its floor.
  * One PSUM tile = two 2KB banks = 8 pairs; 16 matmuls accumulate into
    it, then one FD=512 copy per half moves the useful quadrants to SBUF
    (fp32->fp16), split 1:1 across DVE and ACT.  Big copies amortize the
    ~120-220 cycle per-op overhead.
  * Output: gram tiles are [p, g, q] over 128-pair groups (4 chunks);
    the strict upper triangle ships as a 75% block cover (rows 0:32 x
    cols 0:64 full + rows 32:64 x cols 32:64) = 2 contiguous DMAs per
    half per group -> 16 output DMAs total, ~0.6us trigger cost each.
  * Output DMAs ride the ACT HWDGE ring (inputs ride SP) so the two
    FIFOs never block each other.  Host reassembles + gathers the
    triangle (fixed fancy index) and casts to f32.
"""

import os
import sys

import numpy as np

for _p in ("/opt/trn_rl_repo", "/root/.axon_site/_ro/trn_rl_repo"):
    if os.path.isdir(_p) and _p not in sys.path:
        sys.path.insert(0, _p)

import bass_rust  # noqa: E402
import ml_dtypes  # noqa: E402
from concourse import bacc, bass, mybir, tile  # noqa: E402
from concourse.bass_utils import run_bass_kernel_spmd  # noqa: E402

B, F, D = 8192, 64, 256
N_CORES = 8
B_CORE = B // N_CORES            # 1024
TOT_PAIRS = B_CORE // 2          # 512 pairs per core
N_CHUNKS = 16                    # 32 pairs each
N_GROUPS = 8                     # 2 chunks = 64 pairs per output group
GROUP_PAIRS = TOT_PAIRS // N_GROUPS
CHUNKS_PER_GROUP = N_CHUNKS // N_GROUPS

FP16 = mybir.dt.float16
FP8 = mybir.dt.float8e3
FP32 = mybir.dt.float32

_cache = {}


def _dep(a, b, sync, reason):
    bass_rust.add_dep_helper(a.ins, b.ins, sync=sync, reason=reason)


def _build():
    nc = bacc.Bacc()
    # [d, pair, half, f] per k-block; kb0 fp16, kb1 fp8 e3m4
    xt16 = nc.declare_dram_parameter(
        "xt16", [128, TOT_PAIRS, 2, F], FP16, isOutput=False
    )
    xt8 = nc.declare_dram_parameter(
        "xt8", [128, TOT_PAIRS, 2, F], FP8, isOutput=False
    )
    # Block cover of the strict upper triangle, [half, group, r, c, q]:
    # o1 = rows 0:32 x cols 0:64, o2 = rows 32:64 x cols 32:64.
    o1 = nc.declare_dram_parameter(
        "o1", [2, N_GROUPS, 32, F, GROUP_PAIRS], FP16, isOutput=True
    )
    o2 = nc.declare_dram_parameter(
        "o2", [2, N_GROUPS, 32, 32, GROUP_PAIRS], FP16, isOutput=True
    )

    with tile.TileContext(nc) as tc:
        with (
            tc.tile_pool(name="x16", bufs=6) as x16pool,
            tc.tile_pool(name="x8", bufs=6) as x8pool,
            tc.tile_pool(name="gram", bufs=4) as gpool,
            tc.tile_pool(name="ps", bufs=4, space=bass.MemorySpace.PSUM) as pspool,
        ):
            def flush_output(gi, gram):
                # SWDGE (GpSimd-issued) keeps output triggers off the
                # ACT/SP FIFOs: HWDGE triggers block their engine on
                # shared DMA lane-completion semaphores, stalling the
                # copies queued behind them.  GpSimd is otherwise idle.
                for h in range(2):
                    nc.gpsimd.dma_start(
                        out=o1[h, gi], in_=gram[64 * h : 64 * h + 32, :, :]
                    )
                    nc.gpsimd.dma_start(
                        out=o2[h, gi],
                        in_=gram[64 * h + 32 : 64 * h + 64, 32:64, :],
                    )

            pending = None
            for gi in range(N_GROUPS):
                # Own-half Gram rows for the whole group, [p, g, q]:
                # partition p<64 = sample 2q row p (cols g), p>=64 =
                # sample 2q+1 row p-64.  g outer / q inner keeps both the
                # copy reads and the output DMAs on >=1KB contiguous runs.
                gram = gpool.tile([128, F, GROUP_PAIRS], FP16, tag="gram")
                for ci in range(CHUNKS_PER_GROUP):
                    p0 = gi * GROUP_PAIRS + ci * 32
                    xtile16 = x16pool.tile([128, 32, 2, F], FP16, tag="x16")
                    xtile8 = x8pool.tile([128, 32, 2, F], FP8, tag="x8")
                    if gi == 0 and ci == 0:
                        # Split the very first chunk's loads so the PE can
                        # start on the first 16 pairs ~2us sooner.
                        for lo, hi in ((0, 16), (16, 32)):
                            nc.sync.dma_start(
                                out=xtile16[:, lo:hi, :, :],
                                in_=xt16[:, p0 + lo : p0 + hi, :, :],
                            )
                            nc.sync.dma_start(
                                out=xtile8[:, lo:hi, :, :],
                                in_=xt8[:, p0 + lo : p0 + hi, :, :],
                            )
                    else:
                        nc.sync.dma_start(
                            out=xtile16[:], in_=xt16[:, p0 : p0 + 32, :, :]
                        )
                        nc.sync.dma_start(
                            out=xtile8[:], in_=xt8[:, p0 : p0 + 32, :, :]
                        )

                    for b in range(4):
                        # One PSUM tile = two 2KB banks = 8 pairs.
                        # start=True zeroes a whole bank, so each bank's
                        # first matmul carries it and orders before that
                        # bank's other writers.
                        ps = pspool.tile([128, 8, 2, F], FP32, tag="ps")
                        mms = []
                        for kb in range(2):
                            xk = xtile16 if kb == 0 else xtile8
                            for j in range(8):
                                q = 8 * b + j
                                s = xk[:, q, :, :]   # [128, 2, 64]
                                mms.append(
                                    nc.tensor.matmul(
                                        ps[:, j, :, :],
                                        s,
                                        s,
                                        start=(kb == 0 and j % 4 == 0),
                                        stop=(kb == 1 and j % 4 == 3),
                                        skip_group_check=True,
                                    )
                                )
                        for bank in range(2):
                            zmm = mms[4 * bank]
                            for kb in range(2):
                                for j in range(4):
                                    idx = kb * 8 + 4 * bank + j
                                    if idx != 4 * bank:
                                        _dep(mms[idx], zmm, False, "bank zero order")
                        # Useful quadrants only, transposed to [p, g, j];
                        # DVE takes the A half, ACT the B half.
                        qg = ci * 32 + 8 * b
                        nc.vector.tensor_copy(
                            gram[0:64, :, qg : qg + 8],
                            ps[0:64, :, 0, :].transpose([0, 2, 1]),
                        )
                        nc.scalar.copy(
                            gram[64:128, :, qg : qg + 8],
                            ps[64:128, :, 1, :].transpose([0, 2, 1]),
                        )

                    if ci == 0 and pending is not None:
                        # Emit the previous group's output triggers only
                        # after this group's first chunk of copies: the
                        # trigger instruction blocks its engine on DMA
                        # lane-completion semaphores, so it must reach the
                        # ACT FIFO with those already satisfied.
                        flush_output(*pending)
                        pending = None
                pending = (gi, gram)
            flush_output(*pending)
    nc.compile()
    return nc


def _get_nc():
    if "nc" not in _cache:
        _cache["nc"] = _build()
    return _cache["nc"]


def make_in_maps(inputs: np.ndarray) -> list:
    """Per-core input dicts: [d, pair, h, f] X^T slices, fp16 + fp8 k-blocks."""
    in_maps = []
    for core in range(N_CORES):
        xc = inputs[core * B_CORE : (core + 1) * B_CORE]
        # [pair, h, f, d] -> [d, pair, h, f]
        xp = xc.reshape(TOT_PAIRS, 2, F, D)
        xt16 = np.ascontiguousarray(
            xp[:, :, :, :128].transpose(3, 0, 1, 2)
        ).astype(np.float16)
        xt8 = np.ascontiguousarray(
            xp[:, :, :, 128:].transpose(3, 0, 1, 2)
        ).astype(ml_dtypes.float8_e3m4)
        in_maps.append(
            {
                "xt16": np.ascontiguousarray(xt16),
                "xt8": np.ascontiguousarray(xt8),
            }
        )
    return in_maps


def gather_output(res) -> np.ndarray:
    iu, ju = np.triu_indices(F, k=1)
    outs = []
    for core in range(N_CORES):
        r = res.results[core]
        p1 = np.asarray(r["o1"])  # [2, group, 32, F, q]
        p2 = np.asarray(r["o2"])  # [2, group, 32, 32, q]
        # [group, q, h, f, g] full Gram (lower-left quadrant unused)
        full = np.zeros((N_GROUPS, GROUP_PAIRS, 2, F, F), dtype=np.float16)
        for h in range(2):
            # [group, 32, F, q] -> [group, q, 32, F]
            full[:, :, h, 0:32, :] = p1[h].transpose(0, 3, 1, 2)
            full[:, :, h, 32:64, 32:64] = p2[h].transpose(0, 3, 1, 2)
        gram = full.reshape(B_CORE, F, F)
        outs.append(gram[:, iu, ju])
    return np.concatenate(outs, axis=0).astype(np.float32)


def kernel(inputs: np.ndarray) -> np.ndarray:
    inputs = np.asarray(inputs)
    assert inputs.shape == (B, F, D), inputs.shape

    nc = _get_nc()
    res = run_bass_kernel_spmd(nc, make_in_maps(inputs), list(range(N_CORES)))
    return gather_output(res)
